# revision 1
# baseline (speedup 1.0000x reference)
"""TRN2 Bass kernel for nn_CustomQLoRABigNet: 6 blocks x (3 QLoRA linears),
ReLU, residual, LayerNorm. Data-parallel over 8 NeuronCores (4096 rows each).

On-chip layout: hidden state kept feature-major ("transposed", [k, n]) so all
18 matmuls chain with contraction along SBUF partitions. Weights are
dequantized on-chip per layer: w = (q - 8) * scale, with q shipped as
pre-transposed centered int8 and scales shipped pre-replicated to the matching
[k, o] tile layout (host does pure layout transforms only). Matmuls run in
float32r (fp32 with 12-bit mantissa, full PE rate at N>=256).
"""

import sys

sys.path.insert(0, "/opt/trn_rl_repo")

import numpy as np
import ml_dtypes

import concourse.bass as bass
from concourse import bacc, mybir
import concourse.tile as tile
from concourse.bass_utils import run_bass_kernel_spmd

f32 = mybir.dt.float32
f32r = mybir.dt.float32r
i8 = mybir.dt.int8
bf16 = mybir.dt.bfloat16
AF = mybir.ActivationFunctionType
Alu = mybir.AluOpType

N_CORES = 8
DIM = 1024
KT = 8  # 1024 / 128 partition tiles
NL = 18
RANK = 32
GROUP = 16
BATCH = 32768
RPC = BATCH // N_CORES  # rows per core
CHUNK = 1024  # columns (rows of x) processed per weight pass
NT = 512  # matmul moving free dim (one PSUM bank)
EPS = 1e-5


def fp32r_round(a: np.ndarray) -> np.ndarray:
    """Round-to-nearest-even fp32 -> fp32r (low 12 mantissa bits cleared)."""
    u = np.ascontiguousarray(a, dtype=np.float32).view(np.uint32)
    low = u & np.uint32(0xFFF)
    base = u & ~np.uint32(0xFFF)
    lsb = (u >> np.uint32(12)) & np.uint32(1)
    up = (low > 0x800) | ((low == 0x800) & (lsb == 1))
    out = base + np.where(up, np.uint32(0x1000), np.uint32(0)).astype(np.uint32)
    return out.view(np.float32)


def build_kernel(rows_per_core: int = RPC, chunk: int = CHUNK, n_layers: int = NL):
    nc = bacc.Bacc()
    n_chunks = rows_per_core // chunk
    ntiles = chunk // NT
    n_blocks = n_layers // 3

    x_d = nc.declare_dram_parameter("x_t", [128, KT, rows_per_core], f32r, False)
    wq_d = nc.declare_dram_parameter("wqc", [n_layers, 128, KT, DIM], i8, False)
    sr_d = nc.declare_dram_parameter("srep", [n_layers, 128, KT, DIM], f32, False)
    la_d = nc.declare_dram_parameter("la_t", [n_layers, 128, KT, RANK], f32r, False)
    lb_d = nc.declare_dram_parameter("lb_t", [n_layers, 128, DIM], f32r, False)
    bi_d = nc.declare_dram_parameter("bias_pp", [128, n_layers, KT], f32, False)
    ga_d = nc.declare_dram_parameter("gamma_pp", [128, 5, KT], f32, False)
    be_d = nc.declare_dram_parameter("beta_pp", [128, 5, KT], f32, False)
    id_d = nc.declare_dram_parameter("ident", [128, 128], f32r, False)
    on_d = nc.declare_dram_parameter("ones", [128, 128], f32r, False)
    z_d = nc.declare_dram_parameter("zeros", [128, 2, NT], f32r, False)
    y_d = nc.declare_dram_parameter("y_t", [128, KT, rows_per_core], f32r, True)

    with tile.TileContext(nc) as tc:
        with (
            tc.tile_pool(name="persist", bufs=1) as pp,
            tc.tile_pool(name="wts", bufs=2) as wp,
            tc.tile_pool(name="work", bufs=2) as sp,
            tc.tile_pool(name="ps", bufs=1, space="PSUM") as ps,
        ):
            # persistent tiles
            h_a = pp.tile([128, KT, chunk], f32r)
            h_b = pp.tile([128, KT, chunk], f32r)
            r_t = pp.tile([128, KT, chunk], f32r)
            bias_t = pp.tile([128, n_layers, KT], f32)
            nc.sync.dma_start(bias_t[:, :, :], bi_d[:, :, :])
            gamma_t = pp.tile([128, 5, KT], f32)
            nc.sync.dma_start(gamma_t[:, :, :], ga_d[:, :, :])
            beta_t = pp.tile([128, 5, KT], f32)
            nc.sync.dma_start(beta_t[:, :, :], be_d[:, :, :])
            ident_t = pp.tile([128, 128], f32r)
            nc.sync.dma_start(ident_t[:, :], id_d[:, :])
            ones_t = pp.tile([128, 128], f32r)
            nc.sync.dma_start(ones_t[:, :], on_d[:, :])
            t_pad = pp.tile([128, 2, NT], f32r)
            nc.sync.dma_start(t_pad[:, :, :], z_d[:, :, :])
            ones_col = ones_t[:, 0:1]
            ones_row = ones_t[0:1, :]

            for c in range(n_chunks):
                ccols = bass.ts(c, chunk)
                h_x = h_a if c % 2 == 0 else h_b
                for kt in range(KT):
                    nc.sync.dma_start(h_x[:, kt, :], x_d[:, kt, ccols])
                nc.vector.tensor_copy(r_t[:, :, :], h_x[:, :, :])

                for l in range(n_layers):
                    blk, j = l // 3, l % 3
                    h_in = h_a if (l + c) % 2 == 0 else h_b
                    h_out = h_b if (l + c) % 2 == 0 else h_a

                    # ---- weight load + dequant ----
                    w_t = wp.tile([128, KT, DIM], f32r, tag="wt")
                    for kt in range(KT):
                        wq_t = wp.tile([128, DIM], i8, tag="wqc")
                        nc.sync.dma_start(wq_t[:, :], wq_d[l, :, kt, :])
                        sr_t = wp.tile([128, DIM], f32, tag="srep")
                        nc.sync.dma_start(sr_t[:, :], sr_d[l, :, kt, :])
                        nc.vector.tensor_mul(w_t[:, kt, :], wq_t[:, :], sr_t[:, :])
                    la_t = wp.tile([128, KT, RANK], f32r, tag="lat")
                    nc.sync.dma_start(la_t[:, :, :], la_d[l, :, :, :])
                    lb_t = wp.tile([128, DIM], f32r, tag="lbt")
                    nc.sync.dma_start(lb_t[:, :], lb_d[l, :, :])

                    for nt in range(ntiles):
                        cols = bass.ts(nt, NT)
                        # ---- LoRA stage 1: t = la @ h ----
                        t_ps = ps.tile([32, NT], f32, tag="t", bufs=2)
                        for kt in range(KT):
                            nc.tensor.matmul(
                                t_ps[:, :],
                                lhsT=la_t[:, kt, :],
                                rhs=h_in[:, kt, cols],
                                start=(kt == 0),
                                stop=(kt == KT - 1),
                            )
                        tbuf = (l * ntiles + nt) % 2
                        nc.vector.tensor_copy(t_pad[0:32, tbuf, :], t_ps[:, :])

                        # ---- base + lora stage2 + residual, per output tile ----
                        ln_here = j == 2 and blk < n_blocks - 1
                        if ln_here:
                            s1p = ps.tile([1, NT], f32, tag="s1", bufs=1)
                            s2p = ps.tile([1, NT], f32, tag="s2", bufs=1)
                        for ot in range(KT):
                            y_ps = ps.tile([128, NT], f32, tag="y", bufs=2)
                            for kt in range(KT):
                                nc.tensor.matmul(
                                    y_ps[:, :],
                                    lhsT=w_t[:, kt, bass.ts(ot, 128)],
                                    rhs=h_in[:, kt, cols],
                                    start=(kt == 0),
                                    stop=False,
                                )
                            nc.tensor.matmul(
                                y_ps[:, :],
                                lhsT=lb_t[:, bass.ts(ot, 128)],
                                rhs=t_pad[:, tbuf, :],
                                start=False,
                                stop=(j != 2),
                            )
                            if j == 2:
                                nc.tensor.matmul(
                                    y_ps[:, :],
                                    lhsT=ident_t[:, :],
                                    rhs=r_t[:, ot, cols],
                                    start=False,
                                    stop=True,
                                )
                            nc.scalar.activation(
                                h_out[:, ot, cols],
                                y_ps[:, :],
                                AF.Relu if j < 2 else AF.Identity,
                                bias=bias_t[:, l, ot : ot + 1],
                            )
                            if ln_here:
                                hsq = sp.tile([128, NT], f32r, tag="hsq")
                                nc.scalar.activation(
                                    hsq[:, :], h_out[:, ot, cols], AF.Square
                                )
                                nc.tensor.matmul(
                                    s1p[:, :], lhsT=ones_col,
                                    rhs=h_out[:, ot, cols],
                                    start=(ot == 0), stop=(ot == KT - 1),
                                )
                                nc.tensor.matmul(
                                    s2p[:, :], lhsT=ones_col, rhs=hsq[:, :],
                                    start=(ot == 0), stop=(ot == KT - 1),
                                )

                        # ---- LayerNorm at block end (blocks 0..4) ----
                        if ln_here:
                            m_sb = sp.tile([1, NT], f32, tag="m", bufs=1)
                            nc.vector.tensor_scalar(
                                m_sb[:, :], s1p[:, :], 1.0 / DIM, None, Alu.mult
                            )
                            var_sb = sp.tile([1, NT], f32, tag="var", bufs=1)
                            nc.vector.tensor_scalar(
                                var_sb[:, :], s2p[:, :], 1.0 / DIM, EPS,
                                Alu.mult, Alu.add,
                            )
                            msq = sp.tile([1, NT], f32, tag="msq", bufs=1)
                            nc.vector.tensor_mul(msq[:, :], m_sb[:, :], m_sb[:, :])
                            nc.vector.tensor_sub(var_sb[:, :], var_sb[:, :], msq[:, :])
                            lnv = sp.tile([1, NT], f32, tag="lnv", bufs=1)
                            nc.scalar.activation(lnv[:, :], var_sb[:, :], AF.Ln)
                            i_sb = sp.tile([1, NT], f32r, tag="isb", bufs=1)
                            nc.scalar.activation(i_sb[:, :], lnv[:, :], AF.Exp, scale=-0.5)
                            mi_sb = sp.tile([1, NT], f32r, tag="misb", bufs=1)
                            nc.vector.tensor_mul(mi_sb[:, :], m_sb[:, :], i_sb[:, :])
                            ib_ps = ps.tile([128, NT], f32, tag="bc", bufs=2)
                            nc.tensor.matmul(
                                ib_ps[:, :], lhsT=ones_row, rhs=i_sb[:, :],
                                start=True, stop=True,
                            )
                            mib_ps = ps.tile([128, NT], f32, tag="bc", bufs=2)
                            nc.tensor.matmul(
                                mib_ps[:, :], lhsT=ones_row, rhs=mi_sb[:, :],
                                start=True, stop=True,
                            )
                            for kt in range(KT):
                                nc.vector.tensor_mul(
                                    h_out[:, kt, cols], h_out[:, kt, cols], ib_ps[:, :]
                                )
                                nc.vector.tensor_sub(
                                    h_out[:, kt, cols], h_out[:, kt, cols], mib_ps[:, :]
                                )
                                nc.scalar.activation(
                                    h_out[:, kt, cols],
                                    h_out[:, kt, cols],
                                    AF.Identity,
                                    bias=beta_t[:, blk, kt : kt + 1],
                                    scale=gamma_t[:, blk, kt : kt + 1],
                                )
                                nc.vector.tensor_copy(
                                    r_t[:, kt, cols], h_out[:, kt, cols]
                                )

                h_fin = h_a if (n_layers + c) % 2 == 0 else h_b
                nc.sync.dma_start(y_d[:, :, ccols], h_fin[:, :, :])

    nc.compile()
    return nc


def prep_inputs(x, wq, scales, bias, lora_a, lora_b, gamma, beta,
                rows_per_core=RPC, n_layers=NL):
    """Host-side pure layout prep; returns per-core input maps."""
    nl = n_layers
    wqc = (wq[:nl].transpose(0, 2, 1).astype(np.int8) - 8)  # [l, k, o] centered
    wqc = wqc.reshape(nl, KT, 128, DIM).transpose(0, 2, 1, 3).copy()  # [l,p,kt,o]

    G = scales[:nl].reshape(nl, DIM, 64)  # [l, o, group]
    p_idx = np.arange(128)[:, None] // GROUP  # [128,1]
    kt_idx = np.arange(KT)[None, :] * (128 // GROUP)  # [1,8]
    gidx = p_idx + kt_idx  # [128, 8] -> group row index
    srep = G.transpose(0, 2, 1)[:, gidx, :].astype(np.float32).copy()  # [l,128,8,o]

    la_t = lora_a[:nl].transpose(0, 2, 1).reshape(nl, KT, 128, RANK)
    la_t = fp32r_round(la_t.transpose(0, 2, 1, 3)).copy()  # [l, p, kt, r]
    lb_small = fp32r_round(lora_b[:nl].transpose(0, 2, 1))  # [l, r, o]
    lb_t = np.zeros((nl, 128, DIM), np.float32)
    lb_t[:, :RANK, :] = lb_small

    bias_pp = bias[:nl].reshape(nl, KT, 128).transpose(2, 0, 1).astype(np.float32).copy()
    gamma_pp = gamma.reshape(5, KT, 128).transpose(2, 0, 1).astype(np.float32).copy()
    beta_pp = beta.reshape(5, KT, 128).transpose(2, 0, 1).astype(np.float32).copy()
    ident = np.eye(128, dtype=np.float32)

    shared = {
        "wqc": wqc, "srep": srep, "la_t": la_t, "lb_t": lb_t,
        "bias_pp": bias_pp, "gamma_pp": gamma_pp, "beta_pp": beta_pp,
        "ident": ident, "ones": np.ones((128, 128), np.float32),
        "zeros": np.zeros((128, 2, NT), np.float32),
    }
    in_maps = []
    for c in range(x.shape[0] // rows_per_core):
        xs = x[c * rows_per_core : (c + 1) * rows_per_core]  # [rows, 1024]
        x_t = fp32r_round(xs.T.reshape(KT, 128, rows_per_core).transpose(1, 0, 2)).copy()
        in_maps.append({"x_t": x_t, **shared})
    return in_maps


def unshard_output(results, rows_per_core=RPC):
    outs = []
    for r in results:
        y_t = np.asarray(r["y_t"]).reshape(128, KT, rows_per_core)
        outs.append(y_t.transpose(2, 1, 0).reshape(rows_per_core, DIM))
    return np.ascontiguousarray(np.concatenate(outs, axis=0), dtype=np.float32)


def kernel(x, wq, scales, bias, lora_a, lora_b, gamma, beta):
    x, wq, scales, bias, lora_a, lora_b, gamma, beta = (
        np.asarray(a) for a in (x, wq, scales, bias, lora_a, lora_b, gamma, beta)
    )
    nc = build_kernel()
    in_maps = prep_inputs(x, wq, scales, bias, lora_a, lora_b, gamma, beta)
    res = run_bass_kernel_spmd(nc, in_maps, list(range(N_CORES)))
    return unshard_output(res.results)



# revision 16
# speedup vs baseline: 1.0670x; 1.0670x over previous
"""TRN2 Bass kernel for nn_CustomQLoRABigNet: 6 blocks x (3 QLoRA linears),
ReLU, residual, LayerNorm. Data-parallel over 8 NeuronCores (4096 rows each).

v2 strategy vs baseline:
- LoRA is folded into the dequantized weight once per layer:
  W_eff^T = (q-8)*s + la^T @ lb^T  (16 contraction-32 matmuls + vector adds),
  eliminating the per-activation-tile LoRA stage1/stage2 matmul streams.
- Everything on-chip is bf16 (fp32 PSUM accumulation), halving SBUF/DMA and
  enabling fast weight loads; hidden state is a single full-width buffer
  [128, 8, 4096] updated in place via per-strip snapshots.
- Residual add is fused into the PSUM evacuation on the vector engine
  (scalar_tensor_tensor: (psum + bias) + r); residual tensors are staged
  through DRAM scratch instead of occupying SBUF.
- Weights are built once per layer (single pass over rows), so dequant DMA
  drops from 360MB to ~72MB per core.
"""

import sys

sys.path.insert(0, "/opt/trn_rl_repo")

import numpy as np
import ml_dtypes

import concourse.bass as bass
from concourse import bacc, mybir
import concourse.tile as tile
from concourse.bass_utils import run_bass_kernel_spmd

f32 = mybir.dt.float32
f32r = mybir.dt.float32r
bf16 = mybir.dt.bfloat16
AF = mybir.ActivationFunctionType
Alu = mybir.AluOpType
BF = ml_dtypes.bfloat16

N_CORES = 8
DIM = 1024
KT = 8  # 1024 / 128 partition tiles
NL = 18
RANK = 32
GROUP = 16
BATCH = 32768
RPC = BATCH // N_CORES  # rows per core
NT = 512  # matmul moving free dim (one PSUM bank of fp32)
EPS = 1e-5


def build_kernel(rows: int = RPC, n_layers: int = NL):
    nc = bacc.Bacc()
    nstrip = rows // NT
    n_blocks = n_layers // 3

    x_d = nc.declare_dram_parameter("x_t", [128, KT, rows], bf16, False)
    wq_d = nc.declare_dram_parameter("wq_b", [n_layers, 128, KT, DIM], bf16, False)
    sr_d = nc.declare_dram_parameter("srep", [n_layers, 128, KT, DIM], f32, False)
    la_d = nc.declare_dram_parameter("la_f", [n_layers, RANK, KT, 128], bf16, False)
    lb_d = nc.declare_dram_parameter("lb_f", [n_layers, RANK, DIM], bf16, False)
    bi_d = nc.declare_dram_parameter("bias_pp", [128, n_layers, KT], f32, False)
    ga_d = nc.declare_dram_parameter("gamma_pp", [128, 5, KT], f32, False)
    be_d = nc.declare_dram_parameter("beta_pp", [128, 5, KT], f32, False)
    on_d = nc.declare_dram_parameter("ones", [128, 128], bf16, False)
    onf_d = nc.declare_dram_parameter("ones_f", [1, 128], f32r, False)
    y_d = nc.declare_dram_parameter("y_t", [128, KT, rows], bf16, True)

    with tile.TileContext(nc) as tc:
        with (
            tc.tile_pool(name="persist", bufs=1) as pp,
            tc.tile_pool(name="wts", bufs=2) as wp,
            tc.tile_pool(name="stage", bufs=2) as hp,
            tc.tile_pool(name="small", bufs=2) as sp,
            tc.tile_pool(name="ps_y", bufs=4, space="PSUM") as psy,
            tc.tile_pool(name="ps_f", bufs=2, space="PSUM") as psf,
            tc.tile_pool(name="ps_s", bufs=2, space="PSUM") as pss,
            tc.tile_pool(name="rdram", bufs=1, space="DRAM") as dr,
        ):
            h_t = pp.tile([128, KT, rows], bf16)
            bias_t = pp.tile([128, n_layers, KT], f32)
            nc.sync.dma_start(bias_t[:, :, :], bi_d[:, :, :])
            gamma_t = pp.tile([128, 5, KT], f32)
            nc.sync.dma_start(gamma_t[:, :, :], ga_d[:, :, :])
            beta_t = pp.tile([128, 5, KT], f32)
            nc.sync.dma_start(beta_t[:, :, :], be_d[:, :, :])
            ones_t = pp.tile([128, 128], bf16)
            nc.sync.dma_start(ones_t[:, :], on_d[:, :])
            ones_col = ones_t[:, 0:1]
            ones_fr = pp.tile([1, 128], f32r)
            nc.sync.dma_start(ones_fr[:, :], onf_d[:, :])
            ones_row = ones_fr[0:1, :]

            # residual ping-pong scratch in DRAM (block b reads r_dram[b%2],
            # its LayerNorm output is written to r_dram[(b+1)%2])
            r_dram = [
                dr.tile([128, KT, rows], bf16, tag=f"r{i}", name=f"r_dram{i}")
                for i in range(2)
            ]

            nc.sync.dma_start(h_t[:, :, :], x_d[:, :, :])

            for l in range(n_layers):
                blk, j = l // 3, l % 3
                ln_here = j == 2 and blk < n_blocks - 1

                # ---- weight build: w_eff = (q-8)*s + la^T @ lb^T  (f32r) ----
                w_t = wp.tile([128, KT, DIM], f32r, tag="we")
                la_t = wp.tile([RANK, KT, 128], bf16, tag="la")
                nc.sync.dma_start(la_t[:, :, :], la_d[l, :, :, :])
                lb_t = wp.tile([RANK, DIM], bf16, tag="lb")
                nc.sync.dma_start(lb_t[:, :], lb_d[l, :, :])
                for kt in range(KT):
                    wq_t = wp.tile([128, DIM], bf16, tag="wq")
                    nc.sync.dma_start(wq_t[:, :], wq_d[l, :, kt, :])
                    sr_t = wp.tile([128, DIM], f32, tag="sr")
                    nc.sync.dma_start(sr_t[:, :], sr_d[l, :, kt, :])
                    nc.vector.tensor_mul(w_t[:, kt, :], wq_t[:, :], sr_t[:, :])
                    for oh in range(2):
                        f_ps = psf.tile([128, NT], f32, tag="fold")
                        nc.tensor.matmul(
                            f_ps[:, :],
                            lhsT=la_t[:, kt, :],
                            rhs=lb_t[:, bass.ts(oh, NT)],
                            start=True,
                            stop=True,
                        )
                        nc.vector.tensor_add(
                            w_t[:, kt, bass.ts(oh, NT)],
                            w_t[:, kt, bass.ts(oh, NT)],
                            f_ps[:, :],
                        )

                # ---- main pass: h[:, :, strip] = layer(h[:, :, strip]) ----
                for s in range(nstrip):
                    scols = bass.ts(s, NT)
                    # snapshot converts bf16 -> f32r: matmul requires matching
                    # operand class (mixed f32r x bf16 fails codegen)
                    hs = hp.tile([128, KT, NT], f32r, tag="hs")
                    nc.vector.tensor_copy(hs[:, :, :], h_t[:, :, scols])
                    if j == 2:
                        r_st = hp.tile([128, KT, NT], bf16, tag="rst", bufs=1)
                        if blk == 0:
                            nc.sync.dma_start(r_st[:, :, :], x_d[:, :, scols])
                        else:
                            nc.sync.dma_start(
                                r_st[:, :, :], r_dram[blk % 2][:, :, scols]
                            )
                    for ot in range(KT):
                        y_ps = psy.tile([128, NT], f32, tag="y")
                        for kt in range(KT):
                            nc.tensor.matmul(
                                y_ps[:, :],
                                lhsT=w_t[:, kt, bass.ts(ot, 128)],
                                rhs=hs[:, kt, :],
                                start=(kt == 0),
                                stop=(kt == KT - 1),
                            )
                        if j < 2:
                            nc.scalar.activation(
                                h_t[:, ot, scols],
                                y_ps[:, :],
                                AF.Relu,
                                bias=bias_t[:, l, ot : ot + 1],
                            )
                        else:
                            # h = (psum + bias) + r, fused on vector
                            nc.vector.scalar_tensor_tensor(
                                h_t[:, ot, scols],
                                y_ps[:, :],
                                bias_t[:, l, ot : ot + 1],
                                r_st[:, ot, :],
                                Alu.add,
                                Alu.add,
                            )

                    # ---- LayerNorm at block end (blocks 0..4) ----
                    if ln_here:
                        s1p = pss.tile([1, NT], f32, tag="st")
                        s2p = pss.tile([1, NT], f32, tag="st")
                        for ot in range(KT):
                            hsq = sp.tile([128, NT], bf16, tag="hsq", bufs=1)
                            nc.scalar.activation(
                                hsq[:, :], h_t[:, ot, scols], AF.Square
                            )
                            nc.tensor.matmul(
                                s1p[:, :], lhsT=ones_col, rhs=h_t[:, ot, scols],
                                start=(ot == 0), stop=(ot == KT - 1),
                            )
                            nc.tensor.matmul(
                                s2p[:, :], lhsT=ones_col, rhs=hsq[:, :],
                                start=(ot == 0), stop=(ot == KT - 1),
                            )
                        m_sb = sp.tile([1, NT], f32, tag="m", bufs=1)
                        nc.vector.tensor_scalar(
                            m_sb[:, :], s1p[:, :], 1.0 / DIM, None, Alu.mult
                        )
                        var_sb = sp.tile([1, NT], f32, tag="var", bufs=1)
                        nc.vector.tensor_scalar(
                            var_sb[:, :], s2p[:, :], 1.0 / DIM, EPS, Alu.mult, Alu.add
                        )
                        msq = sp.tile([1, NT], f32, tag="msq", bufs=1)
                        nc.vector.tensor_mul(msq[:, :], m_sb[:, :], m_sb[:, :])
                        nc.vector.tensor_sub(var_sb[:, :], var_sb[:, :], msq[:, :])
                        lnv = sp.tile([1, NT], f32, tag="lnv", bufs=1)
                        nc.scalar.activation(lnv[:, :], var_sb[:, :], AF.Ln)
                        i_sb = sp.tile([1, NT], f32r, tag="isb", bufs=1)
                        nc.scalar.activation(i_sb[:, :], lnv[:, :], AF.Exp, scale=-0.5)
                        mi_sb = sp.tile([1, NT], f32r, tag="misb", bufs=1)
                        nc.vector.tensor_mul(mi_sb[:, :], m_sb[:, :], i_sb[:, :])
                        ib_ps = psf.tile([128, NT], f32, tag="fold")
                        nc.tensor.matmul(
                            ib_ps[:, :], lhsT=ones_row, rhs=i_sb[:, :],
                            start=True, stop=True,
                        )
                        mib_ps = psf.tile([128, NT], f32, tag="fold")
                        nc.tensor.matmul(
                            mib_ps[:, :], lhsT=ones_row, rhs=mi_sb[:, :],
                            start=True, stop=True,
                        )
                        for kt in range(KT):
                            # single-rounding LayerNorm apply, PSUM-direct
                            tmp = sp.tile([128, NT], f32, tag="lntmp", bufs=1)
                            nc.vector.tensor_mul(
                                tmp[:, :], h_t[:, kt, scols], ib_ps[:, :]
                            )
                            nc.vector.tensor_sub(
                                h_t[:, kt, scols], tmp[:, :], mib_ps[:, :]
                            )
                            nc.scalar.activation(
                                h_t[:, kt, scols],
                                h_t[:, kt, scols],
                                AF.Identity,
                                bias=beta_t[:, blk, kt : kt + 1],
                                scale=gamma_t[:, blk, kt : kt + 1],
                            )
                        nc.sync.dma_start(
                            r_dram[(blk + 1) % 2][:, :, scols], h_t[:, :, scols]
                        )
                    if l == n_layers - 1:
                        nc.sync.dma_start(y_d[:, :, scols], h_t[:, :, scols])

    nc.compile()
    return nc


def prep_inputs(x, wq, scales, bias, lora_a, lora_b, gamma, beta,
                rows_per_core=RPC, n_layers=NL):
    """Host-side pure layout/cast prep; returns per-core input maps."""
    nl = n_layers
    # centered transposed weights: [l, p, kt, o] with k = kt*128 + p
    wqc = (wq[:nl].transpose(0, 2, 1).astype(np.float32) - 8.0)
    wqc = wqc.reshape(nl, KT, 128, DIM).transpose(0, 2, 1, 3).astype(BF).copy()

    # per-group scales replicated to the same [l, p, kt, o] layout
    G = scales[:nl].reshape(nl, DIM, 64)  # [l, o, kgroup]
    p_idx = np.arange(128)[:, None] // GROUP  # [128,1]
    kt_idx = np.arange(KT)[None, :] * (128 // GROUP)  # [1,8]
    gidx = p_idx + kt_idx  # [128, 8]
    srep = G.transpose(0, 2, 1)[:, gidx, :].astype(np.float32).copy()  # [l,128,8,o]

    la_f = lora_a[:nl].reshape(nl, RANK, KT, 128).astype(BF).copy()
    lb_f = lora_b[:nl].transpose(0, 2, 1).astype(BF).copy()  # [l, r, o]

    bias_pp = bias[:nl].reshape(nl, KT, 128).transpose(2, 0, 1).astype(np.float32).copy()
    gamma_pp = gamma.reshape(5, KT, 128).transpose(2, 0, 1).astype(np.float32).copy()
    beta_pp = beta.reshape(5, KT, 128).transpose(2, 0, 1).astype(np.float32).copy()

    shared = {
        "wq_b": wqc, "srep": srep, "la_f": la_f, "lb_f": lb_f,
        "bias_pp": bias_pp, "gamma_pp": gamma_pp, "beta_pp": beta_pp,
        "ones": np.ones((128, 128), BF),
        "ones_f": np.ones((1, 128), np.float32),
    }
    in_maps = []
    for c in range(x.shape[0] // rows_per_core):
        xs = x[c * rows_per_core : (c + 1) * rows_per_core]  # [rows, 1024]
        x_t = xs.T.reshape(KT, 128, rows_per_core).transpose(1, 0, 2).astype(BF).copy()
        in_maps.append({"x_t": x_t, **shared})
    return in_maps


def unshard_output(results, rows_per_core=RPC):
    outs = []
    for r in results:
        y_t = np.asarray(r["y_t"]).reshape(128, KT, rows_per_core)
        outs.append(y_t.transpose(2, 1, 0).reshape(rows_per_core, DIM))
    return np.ascontiguousarray(np.concatenate(outs, axis=0), dtype=np.float32)


def kernel(x, wq, scales, bias, lora_a, lora_b, gamma, beta):
    x, wq, scales, bias, lora_a, lora_b, gamma, beta = (
        np.asarray(a) for a in (x, wq, scales, bias, lora_a, lora_b, gamma, beta)
    )
    nc = build_kernel()
    in_maps = prep_inputs(x, wq, scales, bias, lora_a, lora_b, gamma, beta)
    res = run_bass_kernel_spmd(nc, in_maps, list(range(N_CORES)))
    return unshard_output(res.results)
